# revision 9
# baseline (speedup 1.0000x reference)
"""Binarized MLP (784 -> 1024 -> 512 -> 256 -> 10, sign(W) weights) on 8 TRN2 cores.

Strategy: pure data parallel. The batch (16384) is split into 8 shards of
2048 rows, the small binarized weights are replicated. Host-side prep:
  - weights are binarized (sign), transposed to [in, out], cast to bf16
    (+-1 is exact in bf16),
  - each x shard is transposed to feature-major [784, 2048], cast to bf16,
    and zero-padded to 896 (=7*128) rows so the contraction dim tiles evenly.
On device everything stays in SBUF (about 120KB/partition); each layer is a
tiled matmul with fp32 PSUM accumulation and a fused bias+ReLU (ScalarE)
that also casts back to bf16. Output is produced feature-major [10, 2048]
fp32 and transposed back on the host.
"""

from contextlib import ExitStack

import ml_dtypes
import numpy as np

import concourse.bass as bass
import concourse.mybir as mybir
import concourse.tile as tile
from concourse import bacc
from concourse.bass_utils import run_bass_kernel_spmd

N_CORES = 8
B_FULL = 16384
B = B_FULL // N_CORES  # 2048 rows per core
D_IN = 784
K1 = D_IN  # ragged: 6 full k-tiles + one K=16 tile
NT = 512  # batch tile (one PSUM bank of fp32)
NB = B // NT  # 4

BF16 = mybir.dt.bfloat16
F32 = mybir.dt.float32
npbf16 = ml_dtypes.bfloat16

# (K, O) per layer
LAYER_DIMS = [(K1, 1024), (1024, 512), (512, 256), (256, 10)]


def _ktiles(k):
    """Split contraction dim into (start, size) tiles of <=128."""
    return [(s, min(128, k - s)) for s in range(0, k, 128)]

_prog_cache = {}


def _build_program():
    nc = bacc.Bacc("TRN2", target_bir_lowering=False, debug=False)

    xt_d = nc.dram_tensor("xt", [K1, B], BF16, kind="ExternalInput")
    w_d = [
        nc.dram_tensor(f"w{i + 1}t", [k, o], BF16, kind="ExternalInput")
        for i, (k, o) in enumerate(LAYER_DIMS)
    ]
    # biases laid out [partition, o_tile] (o = o_tile*128 + partition)
    b_d = [
        nc.dram_tensor(f"b{i + 1}r", [min(o, 128), max(1, o // 128)], F32, kind="ExternalInput")
        for i, (_, o) in enumerate(LAYER_DIMS)
    ]
    out_d = nc.dram_tensor("out_t", [10, B], F32, kind="ExternalOutput")

    with tile.TileContext(nc) as tc, ExitStack() as ctx:
        res = ctx.enter_context(tc.tile_pool(name="resident", bufs=1))
        psum = ctx.enter_context(tc.tile_pool(name="psum", bufs=8, space="PSUM"))

        relu = mybir.ActivationFunctionType.Relu
        ident = mybir.ActivationFunctionType.Identity

        # ---- loads, in consumption order. Layer-1 is n-major (batch-slice
        # major), so issue (w1[k], x[k] slice n0) pairs first: the first
        # batch-slice's worth of work unlocks as soon as ~2.5MB has landed
        # instead of waiting for the full 4.4MB of layer-1 inputs.
        x_sb = []
        w_sb = [[] for _ in LAYER_DIMS]
        for kt, (ks, kn) in enumerate(_ktiles(K1)):
            wt = res.tile([kn, 1024], BF16, tag=f"w0_{kt}", name=f"w0_{kt}")
            nc.sync.dma_start(wt[:, :], w_d[0][ks : ks + kn, :])
            w_sb[0].append(wt)
            xtile = res.tile([kn, B], BF16, tag=f"x_{kt}", name=f"x_{kt}")
            nc.sync.dma_start(
                xtile[:, 0:NT], xt_d[ks : ks + kn, 0:NT]
            )
            x_sb.append(xtile)
        for n in range(1, NB):
            for kt, (ks, kn) in enumerate(_ktiles(K1)):
                nc.sync.dma_start(
                    x_sb[kt][:, n * NT : (n + 1) * NT],
                    xt_d[ks : ks + kn, n * NT : (n + 1) * NT],
                )
        for li, (k, o) in enumerate(LAYER_DIMS):
            if li == 0:
                continue
            for kt, (ks, kn) in enumerate(_ktiles(k)):
                wt = res.tile([kn, o], BF16, tag=f"w{li}_{kt}", name=f"w{li}_{kt}")
                nc.sync.dma_start(wt[:, :], w_d[li][ks : ks + kn, :])
                w_sb[li].append(wt)
        b_sb = []
        for li, (_, o) in enumerate(LAYER_DIMS):
            bt = res.tile(
                [min(o, 128), max(1, o // 128)], F32, tag=f"b{li}", name=f"b{li}"
            )
            nc.sync.dma_start(bt[:, :], b_d[li][:, :])
            b_sb.append(bt)

        # ---- layers ----
        acts = x_sb
        for li, (k, o) in enumerate(LAYER_DIMS):
            nk = len(_ktiles(k))
            last = li == len(LAYER_DIMS) - 1
            if last:
                h_tiles = [res.tile([o, B], F32, tag="hout", name="hout")]
            else:
                h_tiles = [
                    res.tile([128, B], BF16, tag=f"h{li}_{ot}", name=f"h{li}_{ot}")
                    for ot in range(o // 128)
                ]
            n_o = max(1, o // 128)
            po = min(o, 128)  # psum partitions (10 for the last layer)
            # layer 1 iterates n-major to match x slice arrival order
            order = (
                [(ot, n) for n in range(NB) for ot in range(n_o)]
                if li == 0
                else [(ot, n) for ot in range(n_o) for n in range(NB)]
            )
            for gi, (ot, n) in enumerate(order):
                ps = psum.tile([po, NT], F32, tag="ps", name="ps")
                for kt in range(nk):
                    nc.tensor.matmul(
                        ps[:, :],
                        w_sb[li][kt][:, ot * po : ot * po + po],
                        acts[kt][:, n * NT : (n + 1) * NT],
                        start=(kt == 0),
                        stop=(kt == nk - 1),
                    )
                dst = h_tiles[0 if last else ot][:, n * NT : (n + 1) * NT]
                if last:
                    # final layer: no relu; split bias-add across ScalarE and
                    # VectorE so the 4 epilogue ops don't serialize on one
                    # engine, and stream each slice out as soon as it's done.
                    if n % 2 == 0:
                        nc.scalar.activation(
                            dst, ps[:, :], ident, bias=b_sb[li][:, 0:1]
                        )
                    else:
                        nc.vector.tensor_scalar_add(dst, ps[:, :], b_sb[li][:, 0:1])
                    nc.sync.dma_start(
                        out_d[:, n * NT : (n + 1) * NT], dst
                    )
                else:
                    nc.scalar.activation(
                        dst, ps[:, :], relu, bias=b_sb[li][:, ot : ot + 1]
                    )
            acts = h_tiles

    nc.compile()
    return nc


def _get_program():
    if "nc" not in _prog_cache:
        _prog_cache["nc"] = _build_program()
    return _prog_cache["nc"]


def _prep_shared(w1, b1, w2, b2, w3, b3, w4, b4):
    ws = [np.asarray(w, np.float32) for w in (w1, w2, w3, w4)]
    bs = [np.asarray(b, np.float32) for b in (b1, b2, b3, b4)]
    shared = {}
    for i, ((k, o), w) in enumerate(zip(LAYER_DIMS, ws)):
        wt = np.ascontiguousarray(np.sign(w).T.astype(npbf16))
        assert wt.shape == (k, o)
        shared[f"w{i + 1}t"] = wt
        b = bs[i]
        if o >= 128:
            br = np.ascontiguousarray(b.reshape(o // 128, 128).T)
        else:
            br = np.ascontiguousarray(b.reshape(o, 1))
        shared[f"b{i + 1}r"] = br
    return shared


def _run(inputs, trace=False, tmpdir=None):
    x = np.asarray(inputs["x"], np.float32)
    assert x.shape == (B_FULL, D_IN), x.shape
    nc = _get_program()
    shared = _prep_shared(
        inputs["w1"], inputs["b1"], inputs["w2"], inputs["b2"],
        inputs["w3"], inputs["b3"], inputs["w4"], inputs["b4"],
    )
    in_maps = []
    for c in range(N_CORES):
        xs = x[c * B : (c + 1) * B]  # [B, 784]
        xt = np.ascontiguousarray(xs.T.astype(npbf16))  # [784, B]
        in_maps.append({"xt": xt, **shared})
    res = run_bass_kernel_spmd(
        nc, in_maps, core_ids=list(range(N_CORES)), trace=trace, tmpdir=tmpdir
    )
    out = np.concatenate(
        [np.ascontiguousarray(r["out_t"].T) for r in res.results], axis=0
    )
    return out.astype(np.float32), res


def kernel(**inputs):
    out, _ = _run(inputs, trace=False)
    return out


# revision 13
# speedup vs baseline: 1.0255x; 1.0255x over previous
"""Binarized MLP (784 -> 1024 -> 512 -> 256 -> 10, sign(W) weights) on 8 TRN2 cores.

Strategy: pure data parallel. The batch (16384) is split into 8 shards of
2048 rows, the small binarized weights are replicated. Host-side prep:
  - weights are binarized (sign), transposed to [in, out], cast to bf16
    (+-1 is exact in bf16),
  - each x shard is transposed to feature-major [784, 2048], cast to bf16,
    and zero-padded to 896 (=7*128) rows so the contraction dim tiles evenly.
On device everything stays in SBUF (about 120KB/partition); each layer is a
tiled matmul with fp32 PSUM accumulation and a fused bias+ReLU (ScalarE)
that also casts back to bf16. Output is produced feature-major [10, 2048]
fp32 and transposed back on the host.
"""

from contextlib import ExitStack

import ml_dtypes
import numpy as np

import concourse.bass as bass
import concourse.mybir as mybir
import concourse.tile as tile
from concourse import bacc
from concourse.bass_utils import run_bass_kernel_spmd

N_CORES = 8
B_FULL = 16384
B = B_FULL // N_CORES  # 2048 rows per core
D_IN = 784
K1 = D_IN  # ragged: 6 full k-tiles + one K=16 tile
NT = 512  # batch tile (one PSUM bank of fp32)
NB = B // NT  # 4

BF16 = mybir.dt.bfloat16
F32 = mybir.dt.float32
npbf16 = ml_dtypes.bfloat16

# (K, O) per layer
LAYER_DIMS = [(K1, 1024), (1024, 512), (512, 256), (256, 10)]


def _ktiles(k):
    """Split contraction dim into (start, size) tiles of <=128."""
    return [(s, min(128, k - s)) for s in range(0, k, 128)]

_prog_cache = {}


def _build_program_raw():
    """Hand-scheduled variant: one bass.Block, explicit semaphores, no
    TileContext (saves Tile's prologue/epilogue barriers and gives exact
    control of DMA issue order). PE work is emitted in waves of 8 PSUM
    groups with the contraction dim outermost inside a wave, so the
    in-order PE stream consumes layer-1 input tiles in DMA arrival order.
    """
    nc = bass.Bass("TRN2", target_bir_lowering=False, debug=False)

    xt_d = nc.dram_tensor("xt", [K1, B], BF16, kind="ExternalInput")
    w_d = [
        nc.dram_tensor(f"w{i + 1}t", [k, o], BF16, kind="ExternalInput")
        for i, (k, o) in enumerate(LAYER_DIMS)
    ]
    b_d = [
        nc.dram_tensor(
            f"b{i + 1}r", [min(o, 128), max(1, o // 128)], F32, kind="ExternalInput"
        )
        for i, (_, o) in enumerate(LAYER_DIMS)
    ]
    out_d = nc.dram_tensor("out_t", [10, B], F32, kind="ExternalOutput")

    relu = mybir.ActivationFunctionType.Relu
    ident = mybir.ActivationFunctionType.Identity
    H = B // 2
    kt1 = _ktiles(K1)

    # semaphores
    s_b = nc.alloc_semaphore("s_b")
    s_pair = [nc.alloc_semaphore(f"s_pair{k}") for k in range(len(kt1))]
    s_xb = nc.alloc_semaphore("s_xb")
    s_w234 = nc.alloc_semaphore("s_w234")
    pe_sem = nc.alloc_semaphore("pe_sem")
    act_sem = nc.alloc_semaphore("act_sem")
    sc4_sem = nc.alloc_semaphore("sc4_sem")
    v4_sem = nc.alloc_semaphore("v4_sem")
    out_sem = nc.alloc_semaphore("out_sem")

    ctx = ExitStack()
    sb = lambda name, shape, dt: ctx.enter_context(nc.sbuf_tensor(name, shape, dt))

    # SBUF tensors
    xa = [sb(f"xa{k}", [kn, H], BF16) for k, (_, kn) in enumerate(kt1)]
    xb = [sb(f"xb{k}", [kn, H], BF16) for k, (_, kn) in enumerate(kt1)]
    w_sb = []
    for li, (k, o) in enumerate(LAYER_DIMS):
        w_sb.append(
            [sb(f"w{li}_{kt}", [kn, o], BF16) for kt, (_, kn) in enumerate(_ktiles(k))]
        )
    b_sb = [
        sb(f"b{li}", [min(o, 128), max(1, o // 128)], F32)
        for li, (_, o) in enumerate(LAYER_DIMS)
    ]
    h_sb = [
        [sb(f"h0_{t}", [128, B], BF16) for t in range(8)],
        [sb(f"h1_{t}", [128, B], BF16) for t in range(4)],
        [sb(f"h2_{t}", [128, B], BF16) for t in range(2)],
    ]
    hout = sb("hout", [10, B], F32)
    ps = [
        ctx.enter_context(nc.psum_tensor(f"ps{i}", [128, NT], F32)) for i in range(8)
    ]

    # group schedule: (gi, li, ot, n); layer 1 is half-batch-major
    groups = []
    for h in (0, 1):
        for ot in range(8):
            for n in (2 * h, 2 * h + 1):
                groups.append((0, ot, n))
    for li in (1, 2, 3):
        n_o = max(1, LAYER_DIMS[li][1] // 128)
        for ot in range(n_o):
            for n in range(NB):
                groups.append((li, ot, n))
    assert len(groups) == 60
    # last act index (group order) writing h tile [li][t]; L1-L3 acts all run
    # in group order on ScalarE, so act_sem >= idx+1 means tile ready.
    h_last = {}
    for gi, (li, ot, n) in enumerate(groups):
        if li < 3:
            h_last[(li, ot)] = gi

    def rhs_ap(li, kt, n):
        if li == 0:
            t = xa[kt] if n < 2 else xb[kt]
            c = (n % 2) * NT
            return t[:, c : c + NT]
        return h_sb[li - 1][kt][:, n * NT : (n + 1) * NT]

    def out_part(li):
        return min(LAYER_DIMS[li][1], 128)

    with nc.Block() as block:

        @block.sync
        def _(sync):
            for li in range(4):
                sync.dma_start(b_sb[li][:, :], b_d[li][:, :]).then_inc(s_b, 16)
            for kt, (ks, kn) in enumerate(kt1):
                sync.dma_start(
                    w_sb[0][kt][:, :], w_d[0][ks : ks + kn, :]
                ).then_inc(s_pair[kt], 16)
                sync.dma_start(xa[kt][:, :], xt_d[ks : ks + kn, 0:H]).then_inc(
                    s_pair[kt], 16
                )
            for kt, (ks, kn) in enumerate(kt1):
                sync.dma_start(xb[kt][:, :], xt_d[ks : ks + kn, H:B]).then_inc(
                    s_xb, 16
                )
            # keep full HBM bandwidth on the layer-1 critical path: only
            # issue the later layers' weights once x has fully landed.
            sync.wait_ge(s_xb, 16 * len(kt1))
            for li in (1, 2, 3):
                for kt, (ks, kn) in enumerate(_ktiles(LAYER_DIMS[li][0])):
                    sync.dma_start(
                        w_sb[li][kt][:, :], w_d[li][ks : ks + kn, :]
                    ).then_inc(s_w234, 16)
            # stream the 4 output slices out as their epilogues finish
            for i, (sem, v) in enumerate(
                [(sc4_sem, 1), (v4_sem, 1), (sc4_sem, 2), (v4_sem, 2)]
            ):
                sync.wait_ge(sem, v)
                sync.dma_start(
                    out_d[:, i * NT : (i + 1) * NT],
                    hout[:, i * NT : (i + 1) * NT],
                ).then_inc(out_sem, 16)
            sync.wait_ge(out_sem, 64)

        @block.tensor
        def _(pe):
            for wstart in range(0, 60, 8):
                wave = list(range(wstart, min(wstart + 8, 60)))
                li = groups[wave[0]][0]
                nk = len(_ktiles(LAYER_DIMS[li][0]))
                po = out_part(li)
                for kt in range(nk):
                    # input-arrival waits (first wave of each input phase)
                    if wstart == 0:
                        pe.wait_ge(s_pair[kt], 32)
                    if wstart == 16 and kt == 0:
                        pe.wait_ge(s_xb, 16 * len(kt1))
                    if wstart == 32:
                        if kt == 0:
                            pe.wait_ge(s_w234, 16 * 14)
                        pe.wait_ge(act_sem, h_last[(0, kt)] + 1)
                    if wstart == 48 and groups[wave[0]][0] == 2:
                        pe.wait_ge(act_sem, h_last[(1, kt)] + 1)
                    if wstart == 56:
                        pe.wait_ge(act_sem, h_last[(2, kt)] + 1)
                    for gi in wave:
                        gli, ot, n = groups[gi]
                        assert gli == li
                        if kt == 0 and gi >= 8:
                            # psum bank reuse: previous occupant's act done
                            pe.wait_ge(act_sem, min(gi - 7, 56))
                        mm = pe.matmul(
                            ps[gi % 8][0:po, :],
                            w_sb[li][kt][:, ot * po : ot * po + po],
                            rhs_ap(li, kt, n),
                            start=(kt == 0),
                            stop=(kt == nk - 1),
                        )
                        if kt == nk - 1:
                            mm.then_inc(pe_sem)

        @block.scalar
        def _(sc):
            sc.wait_ge(s_b, 64)
            for gi, (li, ot, n) in enumerate(groups):
                if li == 3 and n % 2 == 1:
                    continue  # VectorE handles odd output slices
                po = out_part(li)
                sc.wait_ge(pe_sem, gi + 1)
                if li < 3:
                    ins = sc.activation(
                        h_sb[li][ot][:, n * NT : (n + 1) * NT],
                        ps[gi % 8][0:po, :],
                        relu,
                        bias=b_sb[li][:, ot : ot + 1],
                    )
                    ins.then_inc(act_sem)
                else:
                    ins = sc.activation(
                        hout[:, n * NT : (n + 1) * NT],
                        ps[gi % 8][0:po, :],
                        ident,
                        bias=b_sb[li][:, 0:1],
                    )
                    ins.then_inc(sc4_sem)

        @block.vector
        def _(v):
            for gi, (li, ot, n) in enumerate(groups):
                if not (li == 3 and n % 2 == 1):
                    continue
                po = out_part(li)
                v.wait_ge(pe_sem, gi + 1)
                v.tensor_scalar_add(
                    hout[:, n * NT : (n + 1) * NT],
                    ps[gi % 8][0:po, :],
                    b_sb[li][:, 0:1],
                ).then_inc(v4_sem)

    ctx.close()
    return nc


def _build_program():
    nc = bacc.Bacc("TRN2", target_bir_lowering=False, debug=False)

    xt_d = nc.dram_tensor("xt", [K1, B], BF16, kind="ExternalInput")
    w_d = [
        nc.dram_tensor(f"w{i + 1}t", [k, o], BF16, kind="ExternalInput")
        for i, (k, o) in enumerate(LAYER_DIMS)
    ]
    # biases laid out [partition, o_tile] (o = o_tile*128 + partition)
    b_d = [
        nc.dram_tensor(f"b{i + 1}r", [min(o, 128), max(1, o // 128)], F32, kind="ExternalInput")
        for i, (_, o) in enumerate(LAYER_DIMS)
    ]
    out_d = nc.dram_tensor("out_t", [10, B], F32, kind="ExternalOutput")

    with tile.TileContext(nc) as tc, ExitStack() as ctx:
        res = ctx.enter_context(tc.tile_pool(name="resident", bufs=1))
        psum = ctx.enter_context(tc.tile_pool(name="psum", bufs=8, space="PSUM"))

        relu = mybir.ActivationFunctionType.Relu
        ident = mybir.ActivationFunctionType.Identity

        # ---- loads, in consumption order.
        # Biases go FIRST: the very first ACT needs b1, and a late bias would
        # pin every PSUM bank (no group can retire) and stall the PE.
        b_sb = []
        for li, (_, o) in enumerate(LAYER_DIMS):
            bt = res.tile(
                [min(o, 128), max(1, o // 128)], F32, tag=f"b{li}", name=f"b{li}"
            )
            nc.sync.dma_start(bt[:, :], b_d[li][:, :])
            b_sb.append(bt)
        # Layer-1 inputs stream as (w1[k], x-half-A[k]) pairs; x is split into
        # two [kn, B/2] half-tiles so the first half-batch of layer-1 work
        # rate-matches DMA arrival (8 in-flight groups consume one pair's
        # worth of matmuls in about the time the next pair lands).
        x_sb = []  # per k: [xa, xb]
        w_sb = [[] for _ in LAYER_DIMS]
        H = B // 2
        for kt, (ks, kn) in enumerate(_ktiles(K1)):
            wt = res.tile([kn, 1024], BF16, tag=f"w0_{kt}", name=f"w0_{kt}")
            nc.sync.dma_start(wt[:, :], w_d[0][ks : ks + kn, :])
            w_sb[0].append(wt)
            xa = res.tile([kn, H], BF16, tag=f"xa_{kt}", name=f"xa_{kt}")
            nc.sync.dma_start(xa[:, :], xt_d[ks : ks + kn, 0:H])
            x_sb.append([xa])
        for kt, (ks, kn) in enumerate(_ktiles(K1)):
            xb = res.tile([kn, H], BF16, tag=f"xb_{kt}", name=f"xb_{kt}")
            nc.sync.dma_start(xb[:, :], xt_d[ks : ks + kn, H:B])
            x_sb[kt].append(xb)
        for li, (k, o) in enumerate(LAYER_DIMS):
            if li == 0:
                continue
            for kt, (ks, kn) in enumerate(_ktiles(k)):
                wt = res.tile([kn, o], BF16, tag=f"w{li}_{kt}", name=f"w{li}_{kt}")
                nc.sync.dma_start(wt[:, :], w_d[li][ks : ks + kn, :])
                w_sb[li].append(wt)

        # ---- layers ----
        acts = x_sb
        for li, (k, o) in enumerate(LAYER_DIMS):
            nk = len(_ktiles(k))
            last = li == len(LAYER_DIMS) - 1
            if last:
                h_tiles = [res.tile([o, B], F32, tag="hout", name="hout")]
            else:
                h_tiles = [
                    res.tile([128, B], BF16, tag=f"h{li}_{ot}", name=f"h{li}_{ot}")
                    for ot in range(o // 128)
                ]
            n_o = max(1, o // 128)
            po = min(o, 128)  # psum partitions (10 for the last layer)
            # layer 1 iterates half-batch-major to match x arrival order
            order = (
                [(ot, n) for h in (0, 1) for ot in range(n_o) for n in (2 * h, 2 * h + 1)]
                if li == 0
                else [(ot, n) for ot in range(n_o) for n in range(NB)]
            )
            for gi, (ot, n) in enumerate(order):
                ps = psum.tile([po, NT], F32, tag="ps", name="ps")
                for kt in range(nk):
                    if li == 0:
                        rhs = x_sb[kt][n // 2][:, (n % 2) * NT : (n % 2 + 1) * NT]
                    else:
                        rhs = acts[kt][:, n * NT : (n + 1) * NT]
                    nc.tensor.matmul(
                        ps[:, :],
                        w_sb[li][kt][:, ot * po : ot * po + po],
                        rhs,
                        start=(kt == 0),
                        stop=(kt == nk - 1),
                    )
                dst = h_tiles[0 if last else ot][:, n * NT : (n + 1) * NT]
                if last:
                    # final layer: no relu; split bias-add across ScalarE and
                    # VectorE so the 4 epilogue ops don't serialize on one
                    # engine, and stream each slice out as soon as it's done.
                    if n % 2 == 0:
                        nc.scalar.activation(
                            dst, ps[:, :], ident, bias=b_sb[li][:, 0:1]
                        )
                    else:
                        nc.vector.tensor_scalar_add(dst, ps[:, :], b_sb[li][:, 0:1])
                    nc.sync.dma_start(
                        out_d[:, n * NT : (n + 1) * NT], dst
                    )
                else:
                    nc.scalar.activation(
                        dst, ps[:, :], relu, bias=b_sb[li][:, ot : ot + 1]
                    )
            acts = h_tiles

    nc.compile()
    return nc


def _get_program():
    if "nc" not in _prog_cache:
        import os

        if os.environ.get("BINMLP_RAW", "0") == "1":
            _prog_cache["nc"] = _build_program_raw()
        else:
            _prog_cache["nc"] = _build_program()
    return _prog_cache["nc"]


def _prep_shared(w1, b1, w2, b2, w3, b3, w4, b4):
    ws = [np.asarray(w, np.float32) for w in (w1, w2, w3, w4)]
    bs = [np.asarray(b, np.float32) for b in (b1, b2, b3, b4)]
    shared = {}
    for i, ((k, o), w) in enumerate(zip(LAYER_DIMS, ws)):
        wt = np.ascontiguousarray(np.sign(w).T.astype(npbf16))
        assert wt.shape == (k, o)
        shared[f"w{i + 1}t"] = wt
        b = bs[i]
        if o >= 128:
            br = np.ascontiguousarray(b.reshape(o // 128, 128).T)
        else:
            br = np.ascontiguousarray(b.reshape(o, 1))
        shared[f"b{i + 1}r"] = br
    return shared


def _run(inputs, trace=False, tmpdir=None):
    x = np.asarray(inputs["x"], np.float32)
    assert x.shape == (B_FULL, D_IN), x.shape
    nc = _get_program()
    shared = _prep_shared(
        inputs["w1"], inputs["b1"], inputs["w2"], inputs["b2"],
        inputs["w3"], inputs["b3"], inputs["w4"], inputs["b4"],
    )
    in_maps = []
    for c in range(N_CORES):
        xs = x[c * B : (c + 1) * B]  # [B, 784]
        xt = np.ascontiguousarray(xs.T.astype(npbf16))  # [784, B]
        in_maps.append({"xt": xt, **shared})
    res = run_bass_kernel_spmd(
        nc, in_maps, core_ids=list(range(N_CORES)), trace=trace, tmpdir=tmpdir
    )
    out = np.concatenate(
        [np.ascontiguousarray(r["out_t"].T) for r in res.results], axis=0
    )
    return out.astype(np.float32), res


def kernel(**inputs):
    out, _ = _run(inputs, trace=False)
    return out


# revision 16
# speedup vs baseline: 1.0759x; 1.0491x over previous
"""Binarized MLP (784 -> 1024 -> 512 -> 256 -> 10, sign(W) weights) on 8 TRN2 cores.

Strategy: pure data parallel. The batch (16384) is split into 8 shards of
2048 rows, the small binarized weights are replicated. Host-side prep:
  - weights are binarized (sign), transposed to [in, out], cast to bf16
    (+-1 is exact in bf16),
  - each x shard is transposed to feature-major [784, 2048], cast to bf16,
    and zero-padded to 896 (=7*128) rows so the contraction dim tiles evenly.
On device everything stays in SBUF (about 120KB/partition); each layer is a
tiled matmul with fp32 PSUM accumulation and a fused bias+ReLU (ScalarE)
that also casts back to bf16. Output is produced feature-major [10, 2048]
fp32 and transposed back on the host.
"""

from contextlib import ExitStack

import ml_dtypes
import numpy as np

import concourse.bass as bass
import concourse.mybir as mybir
import concourse.tile as tile
from concourse import bacc
from concourse.bass_utils import run_bass_kernel_spmd

N_CORES = 8
B_FULL = 16384
B = B_FULL // N_CORES  # 2048 rows per core
D_IN = 784
K1 = D_IN  # ragged: 6 full k-tiles + one K=16 tile
NT = 512  # batch tile (one PSUM bank of fp32)
NB = B // NT  # 4

BF16 = mybir.dt.bfloat16
F32 = mybir.dt.float32
npbf16 = ml_dtypes.bfloat16

# (K, O) per layer
LAYER_DIMS = [(K1, 1024), (1024, 512), (512, 256), (256, 10)]


def _ktiles(k):
    """Split contraction dim into (start, size) tiles of <=128."""
    return [(s, min(128, k - s)) for s in range(0, k, 128)]

_prog_cache = {}


def _build_program_raw():
    """Hand-scheduled variant: one bass.Block, explicit semaphores, no
    TileContext (saves Tile's prologue/epilogue barriers and gives exact
    control of DMA issue order). PE work is emitted in waves of 8 PSUM
    groups with the contraction dim outermost inside a wave, so the
    in-order PE stream consumes layer-1 input tiles in DMA arrival order.
    """
    nc = bacc.Bacc("TRN2", target_bir_lowering=False, debug=False)

    xt_d = nc.dram_tensor("xt", [K1, B], BF16, kind="ExternalInput")
    w_d = [
        nc.dram_tensor(f"w{i + 1}t", [k, o], BF16, kind="ExternalInput")
        for i, (k, o) in enumerate(LAYER_DIMS)
    ]
    b_d = [
        nc.dram_tensor(
            f"b{i + 1}r", [min(o, 128), max(1, o // 128)], F32, kind="ExternalInput"
        )
        for i, (_, o) in enumerate(LAYER_DIMS)
    ]
    out_d = nc.dram_tensor("out_t", [10, B], F32, kind="ExternalOutput")

    relu = mybir.ActivationFunctionType.Relu
    ident = mybir.ActivationFunctionType.Identity
    H = B // 2
    kt1 = _ktiles(K1)

    # semaphores
    s_b = nc.alloc_semaphore("s_b")
    s_pair = [nc.alloc_semaphore(f"s_pair{k}") for k in range(len(kt1))]
    s_xb = nc.alloc_semaphore("s_xb")
    s_w234 = nc.alloc_semaphore("s_w234")
    pe_sem = nc.alloc_semaphore("pe_sem")
    act_sem = nc.alloc_semaphore("act_sem")  # L1+L2 acts (all ScalarE, in order)
    sc3_sem = nc.alloc_semaphore("sc3_sem")  # L3 acts on ScalarE
    v3_sem = nc.alloc_semaphore("v3_sem")  # L3 acts on VectorE
    sc4_sem = nc.alloc_semaphore("sc4_sem")
    v4_sem = nc.alloc_semaphore("v4_sem")
    out_sem = nc.alloc_semaphore("out_sem")

    ctx = ExitStack()
    sb = lambda name, shape, dt: ctx.enter_context(nc.sbuf_tensor(name, shape, dt))

    # SBUF tensors
    xa = [sb(f"xa{k}", [kn, H], BF16) for k, (_, kn) in enumerate(kt1)]
    xb = [sb(f"xb{k}", [kn, H], BF16) for k, (_, kn) in enumerate(kt1)]
    w_sb = []
    for li, (k, o) in enumerate(LAYER_DIMS):
        w_sb.append(
            [sb(f"w{li}_{kt}", [kn, o], BF16) for kt, (_, kn) in enumerate(_ktiles(k))]
        )
    b_sb = [
        sb(f"b{li}", [min(o, 128), max(1, o // 128)], F32)
        for li, (_, o) in enumerate(LAYER_DIMS)
    ]
    h_sb = [
        [sb(f"h0_{t}", [128, B], BF16) for t in range(8)],
        [sb(f"h1_{t}", [128, B], BF16) for t in range(4)],
        [sb(f"h2_{t}", [128, B], BF16) for t in range(2)],
    ]
    hout = sb("hout", [10, B], F32)
    ps = [
        ctx.enter_context(nc.psum_tensor(f"ps{i}", [128, NT], F32)) for i in range(8)
    ]

    # group schedule: (gi, li, ot, n); layer 1 is half-batch-major
    groups = []
    for h in (0, 1):
        for ot in range(8):
            for n in (2 * h, 2 * h + 1):
                groups.append((0, ot, n))
    for li in (1, 2, 3):
        n_o = max(1, LAYER_DIMS[li][1] // 128)
        for ot in range(n_o):
            for n in range(NB):
                groups.append((li, ot, n))
    assert len(groups) == 60
    # last act index (group order) writing h tile [li][t]; L1-L3 acts all run
    # in group order on ScalarE, so act_sem >= idx+1 means tile ready.
    h_last = {}
    for gi, (li, ot, n) in enumerate(groups):
        if li < 3:
            h_last[(li, ot)] = gi

    def rhs_ap(li, kt, n):
        if li == 0:
            t = xa[kt] if n < 2 else xb[kt]
            c = (n % 2) * NT
            return t[:, c : c + NT]
        return h_sb[li - 1][kt][:, n * NT : (n + 1) * NT]

    def out_part(li):
        return min(LAYER_DIMS[li][1], 128)

    # engine assignment for L3/L4 epilogue ops: even gi -> ScalarE, odd -> VectorE
    def on_vector(gi):
        li = groups[gi][0]
        return li >= 2 and gi % 2 == 1

    with nc.Block() as block:

        @block.sync
        def _(sync):
            for kt, (ks, kn) in enumerate(kt1):
                sync.dma_start(
                    w_sb[0][kt][:, :], w_d[0][ks : ks + kn, :]
                ).then_inc(s_pair[kt], 16)
                sync.dma_start(xa[kt][:, :], xt_d[ks : ks + kn, 0:H]).then_inc(
                    s_pair[kt], 16
                )
            for kt, (ks, kn) in enumerate(kt1):
                sync.dma_start(xb[kt][:, :], xt_d[ks : ks + kn, H:B]).then_inc(
                    s_xb, 16
                )
            # keep full HBM bandwidth on the layer-1 critical path: only
            # issue the later layers' weights once x has fully landed.
            sync.wait_ge(s_xb, 16 * len(kt1))
            for li in (1, 2, 3):
                for kt, (ks, kn) in enumerate(_ktiles(LAYER_DIMS[li][0])):
                    sync.dma_start(
                        w_sb[li][kt][:, :], w_d[li][ks : ks + kn, :]
                    ).then_inc(s_w234, 16)
            # two output halves, each as soon as its two epilogue ops finish
            sync.wait_ge(sc4_sem, 1)
            sync.dma_start(out_d[:, 0 : 2 * NT], hout[:, 0 : 2 * NT])._wait_ge(
                v4_sem, 1
            ).then_inc(out_sem, 16)
            sync.wait_ge(sc4_sem, 2)
            sync.dma_start(out_d[:, 2 * NT : B], hout[:, 2 * NT : B])._wait_ge(
                v4_sem, 2
            ).then_inc(out_sem, 16)
            sync.wait_ge(out_sem, 32)

        @block.gpsimd
        def _(g):
            # biases ride the GpSimd SWDGE queue so they don't head-block the
            # layer-1 stream on the sync HWDGE queue
            for li in range(4):
                g.dma_start(b_sb[li][:, :], b_d[li][:, :]).then_inc(s_b, 16)

        @block.tensor
        def _(pe):
            for wstart in range(0, 60, 8):
                wave = list(range(wstart, min(wstart + 8, 60)))
                li = groups[wave[0]][0]
                nk = len(_ktiles(LAYER_DIMS[li][0]))
                po = out_part(li)
                for kt in range(nk):
                    # input-arrival waits (first wave of each input phase)
                    if wstart == 0:
                        pe.wait_ge(s_pair[kt], 32)
                    if wstart == 16 and kt == 0:
                        pe.wait_ge(s_xb, 16 * len(kt1))
                    if wstart == 32:
                        if kt == 0:
                            pe.wait_ge(s_w234, 16 * 14)
                        pe.wait_ge(act_sem, h_last[(0, kt)] + 1)
                    if wstart == 48:
                        pe.wait_ge(act_sem, h_last[(1, kt)] + 1)
                    if wstart == 56:
                        # h3 tile kt ready: its 2 ScalarE + 2 VectorE acts done
                        pe.wait_ge(sc3_sem, 2 * (kt + 1))
                        pe.wait_ge(v3_sem, 2 * (kt + 1))
                    for gi in wave:
                        gli, ot, n = groups[gi]
                        assert gli == li
                        if kt == 0 and 8 <= gi < 56:
                            # psum bank reuse: previous occupant's act done
                            prev = gi - 8
                            if groups[prev][0] < 2:
                                pe.wait_ge(act_sem, prev + 1)
                            elif on_vector(prev):
                                pe.wait_ge(v3_sem, (prev - 48) // 2 + 1)
                            else:
                                pe.wait_ge(sc3_sem, (prev - 48) // 2 + 1)
                        mm = pe.matmul(
                            ps[gi % 8][0:po, :],
                            w_sb[li][kt][:, ot * po : ot * po + po],
                            rhs_ap(li, kt, n),
                            start=(kt == 0),
                            stop=(kt == nk - 1),
                        )
                        if kt == nk - 1:
                            mm.then_inc(pe_sem)

        @block.scalar
        def _(sc):
            sc.wait_ge(s_b, 64)
            for gi, (li, ot, n) in enumerate(groups):
                if on_vector(gi):
                    continue
                po = out_part(li)
                sc.wait_ge(pe_sem, gi + 1)
                if li < 3:
                    ins = sc.activation(
                        h_sb[li][ot][:, n * NT : (n + 1) * NT],
                        ps[gi % 8][0:po, :],
                        relu,
                        bias=b_sb[li][:, ot : ot + 1],
                    )
                    ins.then_inc(sc3_sem if li == 2 else act_sem)
                else:
                    ins = sc.activation(
                        hout[:, n * NT : (n + 1) * NT],
                        ps[gi % 8][0:po, :],
                        ident,
                        bias=b_sb[li][:, 0:1],
                    )
                    ins.then_inc(sc4_sem)

        @block.vector
        def _(v):
            for gi, (li, ot, n) in enumerate(groups):
                if not on_vector(gi):
                    continue
                po = out_part(li)
                v.wait_ge(pe_sem, gi + 1)
                if li == 2:
                    # relu(x + bias) on DVE: tensor_scalar add then max with 0
                    v.tensor_scalar(
                        h_sb[li][ot][:, n * NT : (n + 1) * NT],
                        ps[gi % 8][0:po, :],
                        b_sb[li][:, ot : ot + 1],
                        0.0,
                        mybir.AluOpType.add,
                        mybir.AluOpType.max,
                    ).then_inc(v3_sem)
                else:
                    v.tensor_scalar_add(
                        hout[:, n * NT : (n + 1) * NT],
                        ps[gi % 8][0:po, :],
                        b_sb[li][:, 0:1],
                    ).then_inc(v4_sem)

    ctx.close()
    nc.compile()
    return nc


def _build_program():
    nc = bacc.Bacc("TRN2", target_bir_lowering=False, debug=False)

    xt_d = nc.dram_tensor("xt", [K1, B], BF16, kind="ExternalInput")
    w_d = [
        nc.dram_tensor(f"w{i + 1}t", [k, o], BF16, kind="ExternalInput")
        for i, (k, o) in enumerate(LAYER_DIMS)
    ]
    # biases laid out [partition, o_tile] (o = o_tile*128 + partition)
    b_d = [
        nc.dram_tensor(f"b{i + 1}r", [min(o, 128), max(1, o // 128)], F32, kind="ExternalInput")
        for i, (_, o) in enumerate(LAYER_DIMS)
    ]
    out_d = nc.dram_tensor("out_t", [10, B], F32, kind="ExternalOutput")

    with tile.TileContext(nc) as tc, ExitStack() as ctx:
        res = ctx.enter_context(tc.tile_pool(name="resident", bufs=1))
        psum = ctx.enter_context(tc.tile_pool(name="psum", bufs=8, space="PSUM"))

        relu = mybir.ActivationFunctionType.Relu
        ident = mybir.ActivationFunctionType.Identity

        # ---- loads, in consumption order.
        # Biases go FIRST: the very first ACT needs b1, and a late bias would
        # pin every PSUM bank (no group can retire) and stall the PE.
        b_sb = []
        for li, (_, o) in enumerate(LAYER_DIMS):
            bt = res.tile(
                [min(o, 128), max(1, o // 128)], F32, tag=f"b{li}", name=f"b{li}"
            )
            nc.sync.dma_start(bt[:, :], b_d[li][:, :])
            b_sb.append(bt)
        # Layer-1 inputs stream as (w1[k], x-half-A[k]) pairs; x is split into
        # two [kn, B/2] half-tiles so the first half-batch of layer-1 work
        # rate-matches DMA arrival (8 in-flight groups consume one pair's
        # worth of matmuls in about the time the next pair lands).
        x_sb = []  # per k: [xa, xb]
        w_sb = [[] for _ in LAYER_DIMS]
        H = B // 2
        for kt, (ks, kn) in enumerate(_ktiles(K1)):
            wt = res.tile([kn, 1024], BF16, tag=f"w0_{kt}", name=f"w0_{kt}")
            nc.sync.dma_start(wt[:, :], w_d[0][ks : ks + kn, :])
            w_sb[0].append(wt)
            xa = res.tile([kn, H], BF16, tag=f"xa_{kt}", name=f"xa_{kt}")
            nc.sync.dma_start(xa[:, :], xt_d[ks : ks + kn, 0:H])
            x_sb.append([xa])
        for kt, (ks, kn) in enumerate(_ktiles(K1)):
            xb = res.tile([kn, H], BF16, tag=f"xb_{kt}", name=f"xb_{kt}")
            nc.sync.dma_start(xb[:, :], xt_d[ks : ks + kn, H:B])
            x_sb[kt].append(xb)
        for li, (k, o) in enumerate(LAYER_DIMS):
            if li == 0:
                continue
            for kt, (ks, kn) in enumerate(_ktiles(k)):
                wt = res.tile([kn, o], BF16, tag=f"w{li}_{kt}", name=f"w{li}_{kt}")
                nc.sync.dma_start(wt[:, :], w_d[li][ks : ks + kn, :])
                w_sb[li].append(wt)

        # ---- layers ----
        acts = x_sb
        for li, (k, o) in enumerate(LAYER_DIMS):
            nk = len(_ktiles(k))
            last = li == len(LAYER_DIMS) - 1
            if last:
                h_tiles = [res.tile([o, B], F32, tag="hout", name="hout")]
            else:
                h_tiles = [
                    res.tile([128, B], BF16, tag=f"h{li}_{ot}", name=f"h{li}_{ot}")
                    for ot in range(o // 128)
                ]
            n_o = max(1, o // 128)
            po = min(o, 128)  # psum partitions (10 for the last layer)
            # layer 1 iterates half-batch-major to match x arrival order
            order = (
                [(ot, n) for h in (0, 1) for ot in range(n_o) for n in (2 * h, 2 * h + 1)]
                if li == 0
                else [(ot, n) for ot in range(n_o) for n in range(NB)]
            )
            for gi, (ot, n) in enumerate(order):
                ps = psum.tile([po, NT], F32, tag="ps", name="ps")
                for kt in range(nk):
                    if li == 0:
                        rhs = x_sb[kt][n // 2][:, (n % 2) * NT : (n % 2 + 1) * NT]
                    else:
                        rhs = acts[kt][:, n * NT : (n + 1) * NT]
                    nc.tensor.matmul(
                        ps[:, :],
                        w_sb[li][kt][:, ot * po : ot * po + po],
                        rhs,
                        start=(kt == 0),
                        stop=(kt == nk - 1),
                    )
                dst = h_tiles[0 if last else ot][:, n * NT : (n + 1) * NT]
                if last:
                    # final layer: no relu; split bias-add across ScalarE and
                    # VectorE so the 4 epilogue ops don't serialize on one
                    # engine, and stream each slice out as soon as it's done.
                    if n % 2 == 0:
                        nc.scalar.activation(
                            dst, ps[:, :], ident, bias=b_sb[li][:, 0:1]
                        )
                    else:
                        nc.vector.tensor_scalar_add(dst, ps[:, :], b_sb[li][:, 0:1])
                    nc.sync.dma_start(
                        out_d[:, n * NT : (n + 1) * NT], dst
                    )
                else:
                    nc.scalar.activation(
                        dst, ps[:, :], relu, bias=b_sb[li][:, ot : ot + 1]
                    )
            acts = h_tiles

    nc.compile()
    return nc


def _get_program():
    if "nc" not in _prog_cache:
        import os

        if os.environ.get("BINMLP_RAW", "0") == "1":
            _prog_cache["nc"] = _build_program_raw()
        else:
            _prog_cache["nc"] = _build_program()
    return _prog_cache["nc"]


def _prep_shared(w1, b1, w2, b2, w3, b3, w4, b4):
    ws = [np.asarray(w, np.float32) for w in (w1, w2, w3, w4)]
    bs = [np.asarray(b, np.float32) for b in (b1, b2, b3, b4)]
    shared = {}
    for i, ((k, o), w) in enumerate(zip(LAYER_DIMS, ws)):
        wt = np.ascontiguousarray(np.sign(w).T.astype(npbf16))
        assert wt.shape == (k, o)
        shared[f"w{i + 1}t"] = wt
        b = bs[i]
        if o >= 128:
            br = np.ascontiguousarray(b.reshape(o // 128, 128).T)
        else:
            br = np.ascontiguousarray(b.reshape(o, 1))
        shared[f"b{i + 1}r"] = br
    return shared


def _run(inputs, trace=False, tmpdir=None):
    x = np.asarray(inputs["x"], np.float32)
    assert x.shape == (B_FULL, D_IN), x.shape
    nc = _get_program()
    shared = _prep_shared(
        inputs["w1"], inputs["b1"], inputs["w2"], inputs["b2"],
        inputs["w3"], inputs["b3"], inputs["w4"], inputs["b4"],
    )
    in_maps = []
    for c in range(N_CORES):
        xs = x[c * B : (c + 1) * B]  # [B, 784]
        xt = np.ascontiguousarray(xs.T.astype(npbf16))  # [784, B]
        in_maps.append({"xt": xt, **shared})
    res = run_bass_kernel_spmd(
        nc, in_maps, core_ids=list(range(N_CORES)), trace=trace, tmpdir=tmpdir
    )
    out = np.concatenate(
        [np.ascontiguousarray(r["out_t"].T) for r in res.results], axis=0
    )
    return out.astype(np.float32), res


def kernel(**inputs):
    out, _ = _run(inputs, trace=False)
    return out


# revision 17
# speedup vs baseline: 1.1681x; 1.0857x over previous
"""Binarized MLP (784 -> 1024 -> 512 -> 256 -> 10, sign(W) weights) on 8 TRN2 cores.

Strategy: pure data parallel. The batch (16384) is split into 8 shards of
2048 rows, the small binarized weights are replicated. Host-side prep:
  - weights are binarized (sign), transposed to [in, out], cast to bf16
    (+-1 is exact in bf16),
  - each x shard is transposed to feature-major [784, 2048], cast to bf16,
    and zero-padded to 896 (=7*128) rows so the contraction dim tiles evenly.
On device everything stays in SBUF (about 120KB/partition); each layer is a
tiled matmul with fp32 PSUM accumulation and a fused bias+ReLU (ScalarE)
that also casts back to bf16. Output is produced feature-major [10, 2048]
fp32 and transposed back on the host.
"""

from contextlib import ExitStack

import ml_dtypes
import numpy as np

import concourse.bass as bass
import concourse.mybir as mybir
import concourse.tile as tile
from concourse import bacc
from concourse.bass_utils import run_bass_kernel_spmd

N_CORES = 8
B_FULL = 16384
B = B_FULL // N_CORES  # 2048 rows per core
D_IN = 784
K1 = D_IN  # ragged: 6 full k-tiles + one K=16 tile
NT = 512  # batch tile (one PSUM bank of fp32)
NB = B // NT  # 4

BF16 = mybir.dt.bfloat16
F32 = mybir.dt.float32
npbf16 = ml_dtypes.bfloat16

# (K, O) per layer
LAYER_DIMS = [(K1, 1024), (1024, 512), (512, 256), (256, 10)]


def _ktiles(k):
    """Split contraction dim into (start, size) tiles of <=128."""
    return [(s, min(128, k - s)) for s in range(0, k, 128)]

_prog_cache = {}


def _build_program_raw():
    """Hand-scheduled variant: one bass.Block, explicit semaphores, no
    TileContext (saves Tile's prologue/epilogue barriers and gives exact
    control of DMA issue order). PE work is emitted in waves of 8 PSUM
    groups with the contraction dim outermost inside a wave, so the
    in-order PE stream consumes layer-1 input tiles in DMA arrival order.
    """
    nc = bacc.Bacc("TRN2", target_bir_lowering=False, debug=False)

    xt_d = nc.dram_tensor("xt", [K1, B], BF16, kind="ExternalInput")
    w_d = [
        nc.dram_tensor(f"w{i + 1}t", [k, o], BF16, kind="ExternalInput")
        for i, (k, o) in enumerate(LAYER_DIMS)
    ]
    b_d = [
        nc.dram_tensor(
            f"b{i + 1}r", [min(o, 128), max(1, o // 128)], F32, kind="ExternalInput"
        )
        for i, (_, o) in enumerate(LAYER_DIMS)
    ]
    out_d = nc.dram_tensor("out_t", [10, B], F32, kind="ExternalOutput")

    relu = mybir.ActivationFunctionType.Relu
    ident = mybir.ActivationFunctionType.Identity
    H = B // 2
    kt1 = _ktiles(K1)

    # semaphores
    s_b = nc.alloc_semaphore("s_b")
    s_pair = [nc.alloc_semaphore(f"s_pair{k}") for k in range(len(kt1))]
    s_xb = nc.alloc_semaphore("s_xb")
    s_w234 = nc.alloc_semaphore("s_w234")
    pe_sem = nc.alloc_semaphore("pe_sem")
    act_sem = nc.alloc_semaphore("act_sem")  # L1+L2 acts (all ScalarE, in order)
    sc3_sem = nc.alloc_semaphore("sc3_sem")  # L3 acts on ScalarE
    v3_sem = nc.alloc_semaphore("v3_sem")  # L3 acts on VectorE
    sc4_sem = nc.alloc_semaphore("sc4_sem")
    v4_sem = nc.alloc_semaphore("v4_sem")
    out_sem = nc.alloc_semaphore("out_sem")

    ctx = ExitStack()
    sb = lambda name, shape, dt: ctx.enter_context(nc.sbuf_tensor(name, shape, dt))

    # SBUF tensors
    xa = [sb(f"xa{k}", [kn, H], BF16) for k, (_, kn) in enumerate(kt1)]
    xb = [sb(f"xb{k}", [kn, H], BF16) for k, (_, kn) in enumerate(kt1)]
    w_sb = []
    for li, (k, o) in enumerate(LAYER_DIMS):
        w_sb.append(
            [sb(f"w{li}_{kt}", [kn, o], BF16) for kt, (_, kn) in enumerate(_ktiles(k))]
        )
    b_sb = [
        sb(f"b{li}", [min(o, 128), max(1, o // 128)], F32)
        for li, (_, o) in enumerate(LAYER_DIMS)
    ]
    h_sb = [
        [sb(f"h0_{t}", [128, B], BF16) for t in range(8)],
        [sb(f"h1_{t}", [128, B], BF16) for t in range(4)],
        [sb(f"h2_{t}", [128, B], BF16) for t in range(2)],
    ]
    hout = sb("hout", [10, B], F32)
    ps = [
        ctx.enter_context(nc.psum_tensor(f"ps{i}", [128, NT], F32)) for i in range(8)
    ]

    # group schedule: (gi, li, ot, n); layer 1 is half-batch-major
    groups = []
    for h in (0, 1):
        for ot in range(8):
            for n in (2 * h, 2 * h + 1):
                groups.append((0, ot, n))
    for li in (1, 2, 3):
        n_o = max(1, LAYER_DIMS[li][1] // 128)
        for ot in range(n_o):
            for n in range(NB):
                groups.append((li, ot, n))
    assert len(groups) == 60
    # last act index (group order) writing h tile [li][t]; L1-L3 acts all run
    # in group order on ScalarE, so act_sem >= idx+1 means tile ready.
    h_last = {}
    for gi, (li, ot, n) in enumerate(groups):
        if li < 3:
            h_last[(li, ot)] = gi

    def rhs_ap(li, kt, n):
        if li == 0:
            t = xa[kt] if n < 2 else xb[kt]
            c = (n % 2) * NT
            return t[:, c : c + NT]
        return h_sb[li - 1][kt][:, n * NT : (n + 1) * NT]

    def out_part(li):
        return min(LAYER_DIMS[li][1], 128)

    # engine assignment for L3/L4 epilogue ops: even gi -> ScalarE, odd -> VectorE
    def on_vector(gi):
        li = groups[gi][0]
        return li >= 2 and gi % 2 == 1

    with nc.Block() as block:

        @block.sync
        def _(sync):
            for kt, (ks, kn) in enumerate(kt1):
                sync.dma_start(
                    w_sb[0][kt][:, :], w_d[0][ks : ks + kn, :]
                ).then_inc(s_pair[kt], 16)
                sync.dma_start(xa[kt][:, :], xt_d[ks : ks + kn, 0:H]).then_inc(
                    s_pair[kt], 16
                )
            for kt, (ks, kn) in enumerate(kt1):
                sync.dma_start(xb[kt][:, :], xt_d[ks : ks + kn, H:B]).then_inc(
                    s_xb, 16
                )
            # keep full HBM bandwidth on the layer-1 critical path: only
            # issue the later layers' weights once x has fully landed.
            sync.wait_ge(s_xb, 16 * len(kt1))
            for li in (1, 2, 3):
                for kt, (ks, kn) in enumerate(_ktiles(LAYER_DIMS[li][0])):
                    sync.dma_start(
                        w_sb[li][kt][:, :], w_d[li][ks : ks + kn, :]
                    ).then_inc(s_w234, 16)
            # two output halves, each as soon as its two epilogue ops finish
            sync.wait_ge(sc4_sem, 1)
            sync.dma_start(out_d[:, 0 : 2 * NT], hout[:, 0 : 2 * NT])._wait_ge(
                v4_sem, 1
            ).then_inc(out_sem, 16)
            sync.wait_ge(sc4_sem, 2)
            sync.dma_start(out_d[:, 2 * NT : B], hout[:, 2 * NT : B])._wait_ge(
                v4_sem, 2
            ).then_inc(out_sem, 16)
            sync.wait_ge(out_sem, 32)

        @block.gpsimd
        def _(g):
            # biases ride the GpSimd SWDGE queue so they don't head-block the
            # layer-1 stream on the sync HWDGE queue
            for li in range(4):
                g.dma_start(b_sb[li][:, :], b_d[li][:, :]).then_inc(s_b, 16)

        @block.tensor
        def _(pe):
            def emit_mm(gi, kt):
                li, ot, n = groups[gi]
                nk = len(_ktiles(LAYER_DIMS[li][0]))
                po = out_part(li)
                mm = pe.matmul(
                    ps[gi % 8][0:po, :],
                    w_sb[li][kt][:, ot * po : ot * po + po],
                    rhs_ap(li, kt, n),
                    start=(kt == 0),
                    stop=(kt == nk - 1),
                )
                if kt == nk - 1:
                    mm.then_inc(pe_sem)

            # groups 0-7: k-synchronous wave so the in-order PE stream
            # consumes (w1[k], xa[k]) pairs in DMA arrival order
            for kt in range(len(kt1)):
                pe.wait_ge(s_pair[kt], 32)
                for gi in range(8):
                    emit_mm(gi, kt)
            # groups 8+: per-group contraction-inner, software-pipelined —
            # each group's bank-reuse wait trails its producer by ~8 groups
            for gi in range(8, 60):
                li = groups[gi][0]
                nk = len(_ktiles(LAYER_DIMS[li][0]))
                if gi == 16:
                    pe.wait_ge(s_xb, 16 * len(kt1))
                if gi == 32:
                    pe.wait_ge(s_w234, 16 * 14)
                for kt in range(nk):
                    if gi == 32:
                        pe.wait_ge(act_sem, h_last[(0, kt)] + 1)
                    if gi == 48:
                        pe.wait_ge(act_sem, h_last[(1, kt)] + 1)
                    if gi == 56:
                        # h3 tile kt ready: its 2 ScalarE + 2 VectorE acts done
                        pe.wait_ge(sc3_sem, 2 * (kt + 1))
                        pe.wait_ge(v3_sem, 2 * (kt + 1))
                    if kt == 0 and gi < 56:
                        # psum bank reuse: previous occupant's act done
                        prev = gi - 8
                        if groups[prev][0] < 2:
                            pe.wait_ge(act_sem, prev + 1)
                        elif on_vector(prev):
                            pe.wait_ge(v3_sem, (prev - 48) // 2 + 1)
                        else:
                            pe.wait_ge(sc3_sem, (prev - 48) // 2 + 1)
                    emit_mm(gi, kt)

        @block.scalar
        def _(sc):
            sc.wait_ge(s_b, 64)
            for gi, (li, ot, n) in enumerate(groups):
                if on_vector(gi):
                    continue
                po = out_part(li)
                sc.wait_ge(pe_sem, gi + 1)
                if li < 3:
                    ins = sc.activation(
                        h_sb[li][ot][:, n * NT : (n + 1) * NT],
                        ps[gi % 8][0:po, :],
                        relu,
                        bias=b_sb[li][:, ot : ot + 1],
                    )
                    ins.then_inc(sc3_sem if li == 2 else act_sem)
                else:
                    ins = sc.activation(
                        hout[:, n * NT : (n + 1) * NT],
                        ps[gi % 8][0:po, :],
                        ident,
                        bias=b_sb[li][:, 0:1],
                    )
                    ins.then_inc(sc4_sem)

        @block.vector
        def _(v):
            for gi, (li, ot, n) in enumerate(groups):
                if not on_vector(gi):
                    continue
                po = out_part(li)
                v.wait_ge(pe_sem, gi + 1)
                if li == 2:
                    # relu(x + bias) on DVE: tensor_scalar add then max with 0
                    v.tensor_scalar(
                        h_sb[li][ot][:, n * NT : (n + 1) * NT],
                        ps[gi % 8][0:po, :],
                        b_sb[li][:, ot : ot + 1],
                        0.0,
                        mybir.AluOpType.add,
                        mybir.AluOpType.max,
                    ).then_inc(v3_sem)
                else:
                    v.tensor_scalar_add(
                        hout[:, n * NT : (n + 1) * NT],
                        ps[gi % 8][0:po, :],
                        b_sb[li][:, 0:1],
                    ).then_inc(v4_sem)

    ctx.close()
    nc.compile()
    return nc


def _build_program():
    nc = bacc.Bacc("TRN2", target_bir_lowering=False, debug=False)

    xt_d = nc.dram_tensor("xt", [K1, B], BF16, kind="ExternalInput")
    w_d = [
        nc.dram_tensor(f"w{i + 1}t", [k, o], BF16, kind="ExternalInput")
        for i, (k, o) in enumerate(LAYER_DIMS)
    ]
    # biases laid out [partition, o_tile] (o = o_tile*128 + partition)
    b_d = [
        nc.dram_tensor(f"b{i + 1}r", [min(o, 128), max(1, o // 128)], F32, kind="ExternalInput")
        for i, (_, o) in enumerate(LAYER_DIMS)
    ]
    out_d = nc.dram_tensor("out_t", [10, B], F32, kind="ExternalOutput")

    with tile.TileContext(nc) as tc, ExitStack() as ctx:
        res = ctx.enter_context(tc.tile_pool(name="resident", bufs=1))
        psum = ctx.enter_context(tc.tile_pool(name="psum", bufs=8, space="PSUM"))

        relu = mybir.ActivationFunctionType.Relu
        ident = mybir.ActivationFunctionType.Identity

        # ---- loads, in consumption order.
        # Biases go FIRST: the very first ACT needs b1, and a late bias would
        # pin every PSUM bank (no group can retire) and stall the PE.
        b_sb = []
        for li, (_, o) in enumerate(LAYER_DIMS):
            bt = res.tile(
                [min(o, 128), max(1, o // 128)], F32, tag=f"b{li}", name=f"b{li}"
            )
            nc.sync.dma_start(bt[:, :], b_d[li][:, :])
            b_sb.append(bt)
        # Layer-1 inputs stream as (w1[k], x-half-A[k]) pairs; x is split into
        # two [kn, B/2] half-tiles so the first half-batch of layer-1 work
        # rate-matches DMA arrival (8 in-flight groups consume one pair's
        # worth of matmuls in about the time the next pair lands).
        x_sb = []  # per k: [xa, xb]
        w_sb = [[] for _ in LAYER_DIMS]
        H = B // 2
        for kt, (ks, kn) in enumerate(_ktiles(K1)):
            wt = res.tile([kn, 1024], BF16, tag=f"w0_{kt}", name=f"w0_{kt}")
            nc.sync.dma_start(wt[:, :], w_d[0][ks : ks + kn, :])
            w_sb[0].append(wt)
            xa = res.tile([kn, H], BF16, tag=f"xa_{kt}", name=f"xa_{kt}")
            nc.sync.dma_start(xa[:, :], xt_d[ks : ks + kn, 0:H])
            x_sb.append([xa])
        for kt, (ks, kn) in enumerate(_ktiles(K1)):
            xb = res.tile([kn, H], BF16, tag=f"xb_{kt}", name=f"xb_{kt}")
            nc.sync.dma_start(xb[:, :], xt_d[ks : ks + kn, H:B])
            x_sb[kt].append(xb)
        for li, (k, o) in enumerate(LAYER_DIMS):
            if li == 0:
                continue
            for kt, (ks, kn) in enumerate(_ktiles(k)):
                wt = res.tile([kn, o], BF16, tag=f"w{li}_{kt}", name=f"w{li}_{kt}")
                nc.sync.dma_start(wt[:, :], w_d[li][ks : ks + kn, :])
                w_sb[li].append(wt)

        # ---- layers ----
        acts = x_sb
        for li, (k, o) in enumerate(LAYER_DIMS):
            nk = len(_ktiles(k))
            last = li == len(LAYER_DIMS) - 1
            if last:
                h_tiles = [res.tile([o, B], F32, tag="hout", name="hout")]
            else:
                h_tiles = [
                    res.tile([128, B], BF16, tag=f"h{li}_{ot}", name=f"h{li}_{ot}")
                    for ot in range(o // 128)
                ]
            n_o = max(1, o // 128)
            po = min(o, 128)  # psum partitions (10 for the last layer)
            # layer 1 iterates half-batch-major to match x arrival order
            order = (
                [(ot, n) for h in (0, 1) for ot in range(n_o) for n in (2 * h, 2 * h + 1)]
                if li == 0
                else [(ot, n) for ot in range(n_o) for n in range(NB)]
            )
            for gi, (ot, n) in enumerate(order):
                ps = psum.tile([po, NT], F32, tag="ps", name="ps")
                for kt in range(nk):
                    if li == 0:
                        rhs = x_sb[kt][n // 2][:, (n % 2) * NT : (n % 2 + 1) * NT]
                    else:
                        rhs = acts[kt][:, n * NT : (n + 1) * NT]
                    nc.tensor.matmul(
                        ps[:, :],
                        w_sb[li][kt][:, ot * po : ot * po + po],
                        rhs,
                        start=(kt == 0),
                        stop=(kt == nk - 1),
                    )
                dst = h_tiles[0 if last else ot][:, n * NT : (n + 1) * NT]
                if last:
                    # final layer: no relu; split bias-add across ScalarE and
                    # VectorE so the 4 epilogue ops don't serialize on one
                    # engine, and stream each slice out as soon as it's done.
                    if n % 2 == 0:
                        nc.scalar.activation(
                            dst, ps[:, :], ident, bias=b_sb[li][:, 0:1]
                        )
                    else:
                        nc.vector.tensor_scalar_add(dst, ps[:, :], b_sb[li][:, 0:1])
                    nc.sync.dma_start(
                        out_d[:, n * NT : (n + 1) * NT], dst
                    )
                else:
                    nc.scalar.activation(
                        dst, ps[:, :], relu, bias=b_sb[li][:, ot : ot + 1]
                    )
            acts = h_tiles

    nc.compile()
    return nc


def _get_program():
    if "nc" not in _prog_cache:
        import os

        if os.environ.get("BINMLP_RAW", "0") == "1":
            _prog_cache["nc"] = _build_program_raw()
        else:
            _prog_cache["nc"] = _build_program()
    return _prog_cache["nc"]


def _prep_shared(w1, b1, w2, b2, w3, b3, w4, b4):
    ws = [np.asarray(w, np.float32) for w in (w1, w2, w3, w4)]
    bs = [np.asarray(b, np.float32) for b in (b1, b2, b3, b4)]
    shared = {}
    for i, ((k, o), w) in enumerate(zip(LAYER_DIMS, ws)):
        wt = np.ascontiguousarray(np.sign(w).T.astype(npbf16))
        assert wt.shape == (k, o)
        shared[f"w{i + 1}t"] = wt
        b = bs[i]
        if o >= 128:
            br = np.ascontiguousarray(b.reshape(o // 128, 128).T)
        else:
            br = np.ascontiguousarray(b.reshape(o, 1))
        shared[f"b{i + 1}r"] = br
    return shared


def _run(inputs, trace=False, tmpdir=None):
    x = np.asarray(inputs["x"], np.float32)
    assert x.shape == (B_FULL, D_IN), x.shape
    nc = _get_program()
    shared = _prep_shared(
        inputs["w1"], inputs["b1"], inputs["w2"], inputs["b2"],
        inputs["w3"], inputs["b3"], inputs["w4"], inputs["b4"],
    )
    in_maps = []
    for c in range(N_CORES):
        xs = x[c * B : (c + 1) * B]  # [B, 784]
        xt = np.ascontiguousarray(xs.T.astype(npbf16))  # [784, B]
        in_maps.append({"xt": xt, **shared})
    res = run_bass_kernel_spmd(
        nc, in_maps, core_ids=list(range(N_CORES)), trace=trace, tmpdir=tmpdir
    )
    out = np.concatenate(
        [np.ascontiguousarray(r["out_t"].T) for r in res.results], axis=0
    )
    return out.astype(np.float32), res


def kernel(**inputs):
    out, _ = _run(inputs, trace=False)
    return out


# revision 23
# speedup vs baseline: 1.1736x; 1.0047x over previous
"""Binarized MLP (784 -> 1024 -> 512 -> 256 -> 10, sign(W) weights) on 8 TRN2 cores.

Strategy: pure data parallel. The batch (16384) is split into 8 shards of
2048 rows, the small binarized weights are replicated. Host-side prep:
  - weights are binarized (sign), transposed to [in, out], cast to bf16
    (+-1 is exact in bf16),
  - each x shard is transposed to feature-major [784, 2048], cast to bf16,
    and zero-padded to 896 (=7*128) rows so the contraction dim tiles evenly.
On device everything stays in SBUF (about 120KB/partition); each layer is a
tiled matmul with fp32 PSUM accumulation and a fused bias+ReLU (ScalarE)
that also casts back to bf16. Output is produced feature-major [10, 2048]
fp32 and transposed back on the host.
"""

from contextlib import ExitStack

import ml_dtypes
import numpy as np

import concourse.bass as bass
import concourse.mybir as mybir
import concourse.tile as tile
from concourse import bacc
from concourse.bass_utils import run_bass_kernel_spmd

N_CORES = 8
B_FULL = 16384
B = B_FULL // N_CORES  # 2048 rows per core
D_IN = 784
K1 = D_IN  # ragged: 6 full k-tiles + one K=16 tile
NT = 512  # batch tile (one PSUM bank of fp32)
NB = B // NT  # 4

BF16 = mybir.dt.bfloat16
F32 = mybir.dt.float32
npbf16 = ml_dtypes.bfloat16

# (K, O) per layer
LAYER_DIMS = [(K1, 1024), (1024, 512), (512, 256), (256, 10)]


def _ktiles(k):
    """Split contraction dim into (start, size) tiles of <=128."""
    return [(s, min(128, k - s)) for s in range(0, k, 128)]

_prog_cache = {}


def _build_program_raw():
    """Hand-scheduled variant: one bass.Block, explicit semaphores, no
    TileContext (saves Tile's prologue/epilogue barriers and gives exact
    control of DMA issue order). PE work is emitted in waves of 8 PSUM
    groups with the contraction dim outermost inside a wave, so the
    in-order PE stream consumes layer-1 input tiles in DMA arrival order.
    """
    nc = bacc.Bacc("TRN2", target_bir_lowering=False, debug=False)

    xt_d = nc.dram_tensor("xt", [K1, B], BF16, kind="ExternalInput")
    w_d = [
        nc.dram_tensor(f"w{i + 1}t", [k, o], BF16, kind="ExternalInput")
        for i, (k, o) in enumerate(LAYER_DIMS)
    ]
    b_d = [
        nc.dram_tensor(
            f"b{i + 1}r", [min(o, 128), max(1, o // 128)], F32, kind="ExternalInput"
        )
        for i, (_, o) in enumerate(LAYER_DIMS)
    ]
    out_d = nc.dram_tensor("out_t", [10, B], F32, kind="ExternalOutput")

    relu = mybir.ActivationFunctionType.Relu
    ident = mybir.ActivationFunctionType.Identity
    H = B // 2
    kt1 = _ktiles(K1)

    # semaphores
    s_b = nc.alloc_semaphore("s_b")
    s_pair = [nc.alloc_semaphore(f"s_pair{k}") for k in range(len(kt1))]
    s_xb = nc.alloc_semaphore("s_xb")
    s_w234 = nc.alloc_semaphore("s_w234")
    pe_sem = nc.alloc_semaphore("pe_sem")
    act_sem = nc.alloc_semaphore("act_sem")  # L1+L2 acts (all ScalarE, in order)
    sc3_sem = nc.alloc_semaphore("sc3_sem")  # L3 acts on ScalarE
    v3_sem = nc.alloc_semaphore("v3_sem")  # L3 acts on VectorE
    sc4_sem = nc.alloc_semaphore("sc4_sem")
    v4_sem = nc.alloc_semaphore("v4_sem")
    out_sem = nc.alloc_semaphore("out_sem")

    s_w1b = nc.alloc_semaphore("s_w1b")

    ctx = ExitStack()
    sb = lambda name, shape, dt: ctx.enter_context(nc.sbuf_tensor(name, shape, dt))

    # SBUF tensors. Layer-1 weights are split into column halves (o0-3 /
    # o4-7): the first PE wave only touches o0-3, so only the "a" halves are
    # on the DMA critical path.
    xa = [sb(f"xa{k}", [kn, H], BF16) for k, (_, kn) in enumerate(kt1)]
    xb = [sb(f"xb{k}", [kn, H], BF16) for k, (_, kn) in enumerate(kt1)]
    w1a = [sb(f"w1a{k}", [kn, 512], BF16) for k, (_, kn) in enumerate(kt1)]
    w1b = [sb(f"w1b{k}", [kn, 512], BF16) for k, (_, kn) in enumerate(kt1)]
    w_sb = [None]
    for li, (k, o) in enumerate(LAYER_DIMS):
        if li == 0:
            continue
        w_sb.append(
            [sb(f"w{li}_{kt}", [kn, o], BF16) for kt, (_, kn) in enumerate(_ktiles(k))]
        )
    b_sb = [
        sb(f"b{li}", [min(o, 128), max(1, o // 128)], F32)
        for li, (_, o) in enumerate(LAYER_DIMS)
    ]
    h_sb = [
        [sb(f"h0_{t}", [128, B], BF16) for t in range(8)],
        [sb(f"h1_{t}", [128, B], BF16) for t in range(4)],
        [sb(f"h2_{t}", [128, B], BF16) for t in range(2)],
    ]
    hout = sb("hout", [10, B], F32)
    ps = [
        ctx.enter_context(nc.psum_tensor(f"ps{i}", [128, NT], F32)) for i in range(8)
    ]

    # group schedule: (gi, li, ot, n); layer-1 half A is split into an
    # o0-3 wave then an o4-7 wave (matching the w1a/w1b DMA phases)
    groups = []
    for ob in (0, 1):
        for ot in range(4 * ob, 4 * ob + 4):
            for n in (0, 1):
                groups.append((0, ot, n))
    for ot in range(8):
        for n in (2, 3):
            groups.append((0, ot, n))
    for li in (1, 2, 3):
        n_o = max(1, LAYER_DIMS[li][1] // 128)
        for ot in range(n_o):
            for n in range(NB):
                groups.append((li, ot, n))
    assert len(groups) == 60
    # last act index (group order) writing h tile [li][t]; L1-L3 acts all run
    # in group order on ScalarE, so act_sem >= idx+1 means tile ready.
    h_last = {}
    for gi, (li, ot, n) in enumerate(groups):
        if li < 3:
            h_last[(li, ot)] = gi

    def rhs_ap(li, kt, n):
        if li == 0:
            t = xa[kt] if n < 2 else xb[kt]
            c = (n % 2) * NT
            return t[:, c : c + NT]
        return h_sb[li - 1][kt][:, n * NT : (n + 1) * NT]

    def lhs_ap(li, kt, ot, po):
        if li == 0:
            if ot < 4:
                return w1a[kt][:, ot * 128 : (ot + 1) * 128]
            return w1b[kt][:, (ot - 4) * 128 : (ot - 3) * 128]
        return w_sb[li][kt][:, ot * po : ot * po + po]

    def out_part(li):
        return min(LAYER_DIMS[li][1], 128)

    # engine assignment for L3/L4 epilogue ops: even gi -> ScalarE, odd -> VectorE
    def on_vector(gi):
        li = groups[gi][0]
        return li >= 2 and gi % 2 == 1

    with nc.Block() as block:

        @block.sync
        def _(sync):
            for kt, (ks, kn) in enumerate(kt1):
                sync.dma_start(
                    w1a[kt][:, :], w_d[0][ks : ks + kn, 0:512]
                ).then_inc(s_pair[kt], 16)
                sync.dma_start(xa[kt][:, :], xt_d[ks : ks + kn, 0:H]).then_inc(
                    s_pair[kt], 16
                )
            for kt, (ks, kn) in enumerate(kt1):
                sync.dma_start(
                    w1b[kt][:, :], w_d[0][ks : ks + kn, 512:1024]
                ).then_inc(s_w1b, 16)
            for kt, (ks, kn) in enumerate(kt1):
                sync.dma_start(xb[kt][:, :], xt_d[ks : ks + kn, H:B]).then_inc(
                    s_xb, 16
                )
            # keep full HBM bandwidth on the layer-1 critical path: only
            # issue the later layers' weights once x has fully landed.
            sync.wait_ge(s_xb, 16 * len(kt1))
            for li in (1, 2, 3):
                for kt, (ks, kn) in enumerate(_ktiles(LAYER_DIMS[li][0])):
                    sync.dma_start(
                        w_sb[li][kt][:, :], w_d[li][ks : ks + kn, :]
                    ).then_inc(s_w234, 16)
            # two output halves, each as soon as its two epilogue ops finish
            sync.wait_ge(sc4_sem, 1)
            sync.dma_start(out_d[:, 0 : 2 * NT], hout[:, 0 : 2 * NT])._wait_ge(
                v4_sem, 1
            ).then_inc(out_sem, 16)
            sync.wait_ge(sc4_sem, 2)
            sync.dma_start(out_d[:, 2 * NT : B], hout[:, 2 * NT : B])._wait_ge(
                v4_sem, 2
            ).then_inc(out_sem, 16)
            sync.wait_ge(out_sem, 32)

        @block.gpsimd
        def _(g):
            # biases ride the GpSimd SWDGE queue so they don't head-block the
            # layer-1 stream on the sync HWDGE queue
            for li in range(4):
                g.dma_start(b_sb[li][:, :], b_d[li][:, :]).then_inc(s_b, 16)

        @block.tensor
        def _(pe):
            def emit_mm(gi, kt):
                li, ot, n = groups[gi]
                nk = len(_ktiles(LAYER_DIMS[li][0]))
                po = out_part(li)
                mm = pe.matmul(
                    ps[gi % 8][0:po, :],
                    lhs_ap(li, kt, ot, po),
                    rhs_ap(li, kt, n),
                    start=(kt == 0),
                    stop=(kt == nk - 1),
                )
                if kt == nk - 1:
                    mm.then_inc(pe_sem)

            def reuse_wait(gi):
                # psum bank reuse: previous occupant's act done
                prev = gi - 8
                if groups[prev][0] < 2:
                    pe.wait_ge(act_sem, prev + 1)
                elif on_vector(prev):
                    pe.wait_ge(v3_sem, (prev - 48) // 2 + 1)
                else:
                    pe.wait_ge(sc3_sem, (prev - 48) // 2 + 1)

            # groups 0-7: k-synchronous wave so the in-order PE stream
            # consumes (w1a[k], xa[k]) pairs in DMA arrival order
            for kt in range(len(kt1)):
                pe.wait_ge(s_pair[kt], 32)
                for gi in range(8):
                    emit_mm(gi, kt)
            # groups 8+: per-group contraction-inner, software-pipelined.
            # Bank-reuse waits are hoisted into the PREVIOUS group's stream
            # (they are satisfied ~8 groups early) so each group's first
            # matmul issues back-to-back with its predecessor.
            reuse_wait(8)
            for gi in range(8, 60):
                li = groups[gi][0]
                nk = len(_ktiles(LAYER_DIMS[li][0]))
                if gi == 8:
                    pe.wait_ge(s_w1b, 16 * len(kt1))
                if gi == 16:
                    pe.wait_ge(s_xb, 16 * len(kt1))
                if gi == 32:
                    pe.wait_ge(s_w234, 16 * 14)
                for kt in range(nk):
                    if gi == 32:
                        pe.wait_ge(act_sem, h_last[(0, kt)] + 1)
                    if gi == 48:
                        pe.wait_ge(act_sem, h_last[(1, kt)] + 1)
                    if gi == 56:
                        # h3 tile kt ready: its 2 ScalarE + 2 VectorE acts done
                        pe.wait_ge(sc3_sem, 2 * (kt + 1))
                        pe.wait_ge(v3_sem, 2 * (kt + 1))
                    if kt == 1 and 9 <= gi + 1 < 56:
                        reuse_wait(gi + 1)
                    emit_mm(gi, kt)

        @block.scalar
        def _(sc):
            sc.wait_ge(s_b, 64)
            for gi, (li, ot, n) in enumerate(groups):
                if on_vector(gi):
                    continue
                po = out_part(li)
                sc.wait_ge(pe_sem, gi + 1)
                if li < 3:
                    ins = sc.activation(
                        h_sb[li][ot][:, n * NT : (n + 1) * NT],
                        ps[gi % 8][0:po, :],
                        relu,
                        bias=b_sb[li][:, ot : ot + 1],
                    )
                    ins.then_inc(sc3_sem if li == 2 else act_sem)
                else:
                    ins = sc.activation(
                        hout[:, n * NT : (n + 1) * NT],
                        ps[gi % 8][0:po, :],
                        ident,
                        bias=b_sb[li][:, 0:1],
                    )
                    ins.then_inc(sc4_sem)

        @block.vector
        def _(v):
            for gi, (li, ot, n) in enumerate(groups):
                if not on_vector(gi):
                    continue
                po = out_part(li)
                v.wait_ge(pe_sem, gi + 1)
                if li == 2:
                    # relu(x + bias) on DVE: tensor_scalar add then max with 0
                    v.tensor_scalar(
                        h_sb[li][ot][:, n * NT : (n + 1) * NT],
                        ps[gi % 8][0:po, :],
                        b_sb[li][:, ot : ot + 1],
                        0.0,
                        mybir.AluOpType.add,
                        mybir.AluOpType.max,
                    ).then_inc(v3_sem)
                else:
                    v.tensor_scalar_add(
                        hout[:, n * NT : (n + 1) * NT],
                        ps[gi % 8][0:po, :],
                        b_sb[li][:, 0:1],
                    ).then_inc(v4_sem)

    ctx.close()
    nc.compile()
    return nc


def _build_program():
    nc = bacc.Bacc("TRN2", target_bir_lowering=False, debug=False)

    xt_d = nc.dram_tensor("xt", [K1, B], BF16, kind="ExternalInput")
    w_d = [
        nc.dram_tensor(f"w{i + 1}t", [k, o], BF16, kind="ExternalInput")
        for i, (k, o) in enumerate(LAYER_DIMS)
    ]
    # biases laid out [partition, o_tile] (o = o_tile*128 + partition)
    b_d = [
        nc.dram_tensor(f"b{i + 1}r", [min(o, 128), max(1, o // 128)], F32, kind="ExternalInput")
        for i, (_, o) in enumerate(LAYER_DIMS)
    ]
    out_d = nc.dram_tensor("out_t", [10, B], F32, kind="ExternalOutput")

    with tile.TileContext(nc) as tc, ExitStack() as ctx:
        res = ctx.enter_context(tc.tile_pool(name="resident", bufs=1))
        psum = ctx.enter_context(tc.tile_pool(name="psum", bufs=8, space="PSUM"))

        relu = mybir.ActivationFunctionType.Relu
        ident = mybir.ActivationFunctionType.Identity

        # ---- loads, in consumption order.
        # Biases go FIRST: the very first ACT needs b1, and a late bias would
        # pin every PSUM bank (no group can retire) and stall the PE.
        b_sb = []
        for li, (_, o) in enumerate(LAYER_DIMS):
            bt = res.tile(
                [min(o, 128), max(1, o // 128)], F32, tag=f"b{li}", name=f"b{li}"
            )
            nc.sync.dma_start(bt[:, :], b_d[li][:, :])
            b_sb.append(bt)
        # Layer-1 inputs stream as (w1[k], x-half-A[k]) pairs; x is split into
        # two [kn, B/2] half-tiles so the first half-batch of layer-1 work
        # rate-matches DMA arrival (8 in-flight groups consume one pair's
        # worth of matmuls in about the time the next pair lands).
        x_sb = []  # per k: [xa, xb]
        w_sb = [[] for _ in LAYER_DIMS]
        H = B // 2
        for kt, (ks, kn) in enumerate(_ktiles(K1)):
            wt = res.tile([kn, 1024], BF16, tag=f"w0_{kt}", name=f"w0_{kt}")
            nc.sync.dma_start(wt[:, :], w_d[0][ks : ks + kn, :])
            w_sb[0].append(wt)
            xa = res.tile([kn, H], BF16, tag=f"xa_{kt}", name=f"xa_{kt}")
            nc.sync.dma_start(xa[:, :], xt_d[ks : ks + kn, 0:H])
            x_sb.append([xa])
        for kt, (ks, kn) in enumerate(_ktiles(K1)):
            xb = res.tile([kn, H], BF16, tag=f"xb_{kt}", name=f"xb_{kt}")
            nc.sync.dma_start(xb[:, :], xt_d[ks : ks + kn, H:B])
            x_sb[kt].append(xb)
        for li, (k, o) in enumerate(LAYER_DIMS):
            if li == 0:
                continue
            for kt, (ks, kn) in enumerate(_ktiles(k)):
                wt = res.tile([kn, o], BF16, tag=f"w{li}_{kt}", name=f"w{li}_{kt}")
                nc.sync.dma_start(wt[:, :], w_d[li][ks : ks + kn, :])
                w_sb[li].append(wt)

        # ---- layers ----
        acts = x_sb
        for li, (k, o) in enumerate(LAYER_DIMS):
            nk = len(_ktiles(k))
            last = li == len(LAYER_DIMS) - 1
            if last:
                h_tiles = [res.tile([o, B], F32, tag="hout", name="hout")]
            else:
                h_tiles = [
                    res.tile([128, B], BF16, tag=f"h{li}_{ot}", name=f"h{li}_{ot}")
                    for ot in range(o // 128)
                ]
            n_o = max(1, o // 128)
            po = min(o, 128)  # psum partitions (10 for the last layer)
            # layer 1 iterates half-batch-major to match x arrival order
            order = (
                [(ot, n) for h in (0, 1) for ot in range(n_o) for n in (2 * h, 2 * h + 1)]
                if li == 0
                else [(ot, n) for ot in range(n_o) for n in range(NB)]
            )
            for gi, (ot, n) in enumerate(order):
                ps = psum.tile([po, NT], F32, tag="ps", name="ps")
                for kt in range(nk):
                    if li == 0:
                        rhs = x_sb[kt][n // 2][:, (n % 2) * NT : (n % 2 + 1) * NT]
                    else:
                        rhs = acts[kt][:, n * NT : (n + 1) * NT]
                    nc.tensor.matmul(
                        ps[:, :],
                        w_sb[li][kt][:, ot * po : ot * po + po],
                        rhs,
                        start=(kt == 0),
                        stop=(kt == nk - 1),
                    )
                dst = h_tiles[0 if last else ot][:, n * NT : (n + 1) * NT]
                if last:
                    # final layer: no relu; split bias-add across ScalarE and
                    # VectorE so the 4 epilogue ops don't serialize on one
                    # engine, and stream each slice out as soon as it's done.
                    if n % 2 == 0:
                        nc.scalar.activation(
                            dst, ps[:, :], ident, bias=b_sb[li][:, 0:1]
                        )
                    else:
                        nc.vector.tensor_scalar_add(dst, ps[:, :], b_sb[li][:, 0:1])
                    nc.sync.dma_start(
                        out_d[:, n * NT : (n + 1) * NT], dst
                    )
                else:
                    nc.scalar.activation(
                        dst, ps[:, :], relu, bias=b_sb[li][:, ot : ot + 1]
                    )
            acts = h_tiles

    nc.compile()
    return nc


def _get_program():
    if "nc" not in _prog_cache:
        import os

        if os.environ.get("BINMLP_RAW", "0") == "1":
            _prog_cache["nc"] = _build_program_raw()
        else:
            _prog_cache["nc"] = _build_program()
    return _prog_cache["nc"]


def _prep_shared(w1, b1, w2, b2, w3, b3, w4, b4):
    ws = [np.asarray(w, np.float32) for w in (w1, w2, w3, w4)]
    bs = [np.asarray(b, np.float32) for b in (b1, b2, b3, b4)]
    shared = {}
    for i, ((k, o), w) in enumerate(zip(LAYER_DIMS, ws)):
        wt = np.ascontiguousarray(np.sign(w).T.astype(npbf16))
        assert wt.shape == (k, o)
        shared[f"w{i + 1}t"] = wt
        b = bs[i]
        if o >= 128:
            br = np.ascontiguousarray(b.reshape(o // 128, 128).T)
        else:
            br = np.ascontiguousarray(b.reshape(o, 1))
        shared[f"b{i + 1}r"] = br
    return shared


def _run(inputs, trace=False, tmpdir=None):
    x = np.asarray(inputs["x"], np.float32)
    assert x.shape == (B_FULL, D_IN), x.shape
    nc = _get_program()
    shared = _prep_shared(
        inputs["w1"], inputs["b1"], inputs["w2"], inputs["b2"],
        inputs["w3"], inputs["b3"], inputs["w4"], inputs["b4"],
    )
    in_maps = []
    for c in range(N_CORES):
        xs = x[c * B : (c + 1) * B]  # [B, 784]
        xt = np.ascontiguousarray(xs.T.astype(npbf16))  # [784, B]
        in_maps.append({"xt": xt, **shared})
    res = run_bass_kernel_spmd(
        nc, in_maps, core_ids=list(range(N_CORES)), trace=trace, tmpdir=tmpdir
    )
    out = np.concatenate(
        [np.ascontiguousarray(r["out_t"].T) for r in res.results], axis=0
    )
    return out.astype(np.float32), res


def kernel(**inputs):
    out, _ = _run(inputs, trace=False)
    return out
